# revision 10
# baseline (speedup 1.0000x reference)
"""Trainium2 Bass kernel for nn_DeformableCrossAttention_79551384257136.

Self-contained: takes FULL unsharded inputs, returns FULL output.

Sharding: data-parallel over (batch, query-half): core c handles batch c//2,
query rows [(c%2)*1024, (c%2+1)*1024). Each core sees its batch's full
key/value (4096 x 512) and produces yT [512, 1024] (host re-transposes).

Per-core algorithm:
  1. off/attn GEMM in fp32 on PE (qT chunks as lhsT).
  2. loc indices: ACT sigmoid -> *4095 -> trunc, then corrected EXACTLY to the
     reference's float32 semantics via a host-precomputed threshold table
     (bisected boundaries of m <= sigmoid(x)*4095), gathered with dma_gather
     and a +-15 comparison window.
  3. Sampled K/V rows gathered in bf16 with dma_gather (32 rows/query).
  4. scores: fused DVE tensor_tensor_reduce (q . k * scale + attn_logit).
  5. softmax over the 4 points per head (DVE/ACT, fp32).
  6. Weighted value sum as PE diag-matmuls: psum[c, l] += Vg_p.T @ diag(w_p),
     giving attn_out TRANSPOSED for the proj GEMM.
  7. proj GEMM in bf16 on PE -> yT (+bias), DMA out.
"""

import functools
import os

import numpy as np

import concourse.bass as bass
import concourse.mybir as mybir
import concourse.tile as tile
from concourse import bacc
from concourse.bass_utils import run_bass_kernel_spmd
from concourse.library_config import mlp as _mlp_lib

try:
    import ml_dtypes

    _BF16 = ml_dtypes.bfloat16
except ImportError:  # pragma: no cover
    _BF16 = np.dtype("bfloat16")

F32 = mybir.dt.float32
BF16 = mybir.dt.bfloat16
I16 = mybir.dt.int16

# problem shape (hardcoded per contract)
B, LQ, LK, C, H, P = 4, 2048, 4096, 512, 8, 4
NS = H * P  # 32 sample slots per query
NCORES = 8
LQC = LQ // 2  # queries per core (1024)
NT = LQC // 128  # Lq tiles per core (8)
G = 4  # tiles per proj group
KC = C // 128  # contraction chunks for C (4)
NPC = (C * H) // 128  # proj contraction chunks (32)
WIN = 32  # threshold window entries used (of each 64-entry row)
SCALE = 1.0 / float(np.sqrt(np.float32(C)))
LAST_RES = None


# ----------------------------------------------------------------------------
# threshold table: T[m] = min float32 x with the reference index >= m,
# computed with the same jax that executes the reference. Input-independent.
# ----------------------------------------------------------------------------
def _f32_key(x):
    b = np.asarray(x, np.float32).view(np.uint32).astype(np.int64)
    neg = b >= 0x80000000
    return np.where(neg, 0xFFFFFFFF - b, b + 0x80000000)


def _key_f32(k):
    k = np.asarray(k, np.int64)
    neg = k < 0x80000000
    b = np.where(neg, 0xFFFFFFFF - k, k - 0x80000000).astype(np.uint32)
    return b.view(np.float32)


@functools.lru_cache(maxsize=1)
def _thr_window_table():
    cache = os.path.join(
        os.path.dirname(os.path.abspath(__file__)), f".thr_cache_{LK}.npy"
    )
    T = None
    if os.path.exists(cache):
        try:
            T = np.load(cache)
        except Exception:
            T = None
    if T is None or T.shape != (LK,):
        import jax
        import jax.numpy as jnp

        def _idx(x):
            # eager, op-by-op: must match reference.py's eager dispatch
            # numerics exactly (a fused jit compiles differently here).
            s = jax.nn.sigmoid(x)
            t = s * (LK - 1)
            n = t.astype(jnp.int32)
            return jnp.clip(n, 0, LK - 1)

        m = np.arange(1, LK, dtype=np.int64)  # 1..4095
        lo = np.full(m.shape, _f32_key(np.float32(-40.0)), np.int64)
        hi = np.full(m.shape, _f32_key(np.float32(40.0)), np.int64)
        # invariant: idx(inv(hi)) >= m, idx(inv(lo)) < m
        while True:
            if ((hi - lo) <= 1).all():
                break
            mid = (lo + hi) // 2
            v = np.asarray(_idx(jnp.asarray(_key_f32(mid)))).astype(np.int64)
            ge = v >= m
            hi = np.where(ge, mid, hi)
            lo = np.where(ge, lo, mid)
        T = np.concatenate(
            [[np.float32(-1e30)], _key_f32(hi)]
        ).astype(np.float32)  # T[0] = -inf sentinel, T[1..4095]
        try:
            np.save(cache, T)
        except Exception:
            pass
    # window table: win[m, j] = T_pad[m - 15 + j], j in 0..31; cols 32..63 pad
    Tpad = np.concatenate(
        [
            np.full(15, np.float32(-1e30), np.float32),
            T.astype(np.float32),
            np.full(17, np.float32(1e30), np.float32),
        ]
    )
    idx = np.arange(LK)[:, None] + np.arange(WIN)[None, :]
    win = np.zeros((LK, 64), np.float32)
    win[:, :WIN] = Tpad[idx]
    return T, win


def _searchsorted_loc(x):
    """Reference-exact loc from off values using the threshold table."""
    T, _ = _thr_window_table()
    return np.searchsorted(T[1:], np.asarray(x, np.float32), side="right").astype(
        np.int32
    )


# ----------------------------------------------------------------------------
# bass program (built once; SPMD across 8 cores via per-core in_maps)
# ----------------------------------------------------------------------------
def _build_program():
    nc = bacc.Bacc(
        "TRN2", target_bir_lowering=False, debug=False, num_devices=NCORES
    )
    dt_in = dict(
        qt=([C, LQC], F32),
        qr=([LQC, C], F32),
        kbf=([LK, C], BF16),
        vbf=([LK, C], BF16),
        woa=([C, 2 * NS], F32),
        boa=([1, 2 * NS], F32),
        wp=([C * H, C], BF16),
        bp=([1, C], F32),
        thr=([LK, 64], F32),
        ident=([128, 128], BF16),
    )
    ins = {
        k: nc.dram_tensor(k, s, d, kind="ExternalInput").ap()
        for k, (s, d) in dt_in.items()
    }
    yt = nc.dram_tensor("yt", [C, LQC], F32, kind="ExternalOutput").ap()
    dbg_loc = nc.dram_tensor(
        "dbg_loc", [NT, 128, NS], I16, kind="ExternalOutput"
    ).ap()
    dbg_oa = nc.dram_tensor(
        "dbg_oa", [NT, 128, 2 * NS], F32, kind="ExternalOutput"
    ).ap()

    with tile.TileContext(nc) as tc:
        _body(tc, ins, yt, dbg_loc, dbg_oa)
    nc.compile()
    return nc


def _body(tc, ins, yt, dbg_loc, dbg_oa):
    import os as _os
    from contextlib import ExitStack

    STAGE = int(_os.environ.get("K_STAGE", "99"))

    nc = tc.nc
    with ExitStack() as ctx:
        const = ctx.enter_context(tc.tile_pool(name="const", bufs=1))
        work = ctx.enter_context(tc.tile_pool(name="work", bufs=2))
        gath = ctx.enter_context(tc.tile_pool(name="gath", bufs=1))
        att = ctx.enter_context(tc.tile_pool(name="att", bufs=1))
        psum = ctx.enter_context(tc.tile_pool(name="psum", bufs=2, space="PSUM"))
        dram = ctx.enter_context(tc.tile_pool(name="dram", bufs=2, space="DRAM"))

        nc.gpsimd.load_library(_mlp_lib)

        # ---- constants ----
        woa_sb = const.tile([128, KC, 2 * NS], F32)
        nc.sync.dma_start(
            out=woa_sb, in_=ins["woa"].rearrange("(k p) n -> p k n", p=128)
        )
        boa_sb = const.tile([128, 2 * NS], F32)
        nc.gpsimd.dma_start(
            out=boa_sb,
            in_=bass.AP(
                tensor=ins["boa"].tensor, offset=0, ap=[[0, 128], [1, 2 * NS]]
            ),
        )
        wp_sb = const.tile([128, NPC, C], BF16)
        nc.sync.dma_start(
            out=wp_sb, in_=ins["wp"].rearrange("(k p) n -> p k n", p=128)
        )
        bp_sb = const.tile([128, KC], F32)
        nc.sync.dma_start(
            out=bp_sb,
            in_=bass.AP(
                tensor=ins["bp"].tensor, offset=0, ap=[[1, 128], [128, KC]]
            ),
        )
        id_sb = const.tile([128, 128], BF16)
        nc.sync.dma_start(out=id_sb, in_=ins["ident"])

        attnT = None
        for t in range(NT):
            # ---- per-tile query loads ----
            qt_t = work.tile([128, KC, 128], F32, tag="qt")
            nc.sync.dma_start(
                out=qt_t,
                in_=ins["qt"]
                .rearrange("(k p) l -> p k l", p=128)[:, :, 128 * t : 128 * (t + 1)],
            )
            qr_t = work.tile([128, C], F32, tag="qr")
            nc.sync.dma_start(
                out=qr_t, in_=ins["qr"][128 * t : 128 * (t + 1), :]
            )

            # ---- off/attn GEMM (fp32) ----
            oa_ps = psum.tile([128, 2 * NS], F32, tag="oa", space="PSUM")
            for k in range(KC):
                nc.tensor.matmul(
                    oa_ps,
                    qt_t[:, k, :],
                    woa_sb[:, k, :],
                    start=(k == 0),
                    stop=(k == KC - 1),
                )
            oa_sb = work.tile([128, 2 * NS], F32, tag="oa_sb")
            nc.vector.tensor_add(oa_sb, oa_ps, boa_sb)
            nc.sync.dma_start(out=dbg_oa[t], in_=oa_sb)

            # ---- approximate index ----
            if STAGE < 3:
                continue
            eneg = work.tile([128, NS], F32, tag="eneg")
            nc.scalar.activation(
                eneg, oa_sb[:, 0:NS], mybir.ActivationFunctionType.Exp,
                scale=-1.0,
            )
            d1 = work.tile([128, NS], F32, tag="d1")
            nc.vector.tensor_scalar_add(d1, eneg, 1.0)
            sg = work.tile([128, NS], F32, tag="sg")
            nc.vector.reciprocal(sg, d1)
            tt = work.tile([128, NS], F32, tag="tt")
            nc.vector.tensor_scalar_mul(tt, sg, float(LK - 1))
            nh16 = work.tile([128, NS], I16, tag="nh16")
            nc.vector.tensor_copy(nh16, tt)
            nhf = work.tile([128, NS], F32, tag="nhf")
            nc.vector.tensor_copy(nhf, nh16)

            # ---- rewrap nh16 -> idx for threshold gather ----
            if STAGE < 4:
                continue
            idx_thr = _rewrap(nc, work, dram, nh16, "thr")

            thr_sb = gath.tile([128, NS, 64], F32, tag="thrw")
            nc.gpsimd.dma_gather(
                thr_sb[:], ins["thr"][:], idx_thr[:], 128 * NS, 128 * NS, 64,
                single_packet=False,
            )

            # ---- correct: n = nh - 16 + sum_j [win_j <= x] ----
            if STAGE < 5:
                continue
            ge = work.tile([128, NS, WIN], F32, tag="ge")
            nc.vector.tensor_tensor(
                out=ge,
                in0=thr_sb[:, :, 0:WIN],
                in1=oa_sb[:, 0:NS].to_broadcast([128, NS, WIN]),
                op=mybir.AluOpType.is_le,
            )
            wsum = work.tile([128, NS], F32, tag="wsum")
            nc.vector.tensor_reduce(
                wsum, ge, axis=mybir.AxisListType.X, op=mybir.AluOpType.add
            )
            nf = work.tile([128, NS], F32, tag="nf")
            nc.vector.tensor_scalar_add(nf, wsum, -16.0)
            nc.vector.tensor_add(nf, nf, nhf)
            n16 = work.tile([128, NS], I16, tag="n16")
            nc.vector.tensor_copy(n16, nf)
            nc.sync.dma_start(out=dbg_loc[t], in_=n16)

            # ---- rewrap n16 -> idx for K/V gathers ----
            idx_kv = _rewrap(nc, work, dram, n16, "kv")

            # ---- gather K/V halves (bf16), 2048 rows each ----
            if STAGE < 6:
                continue
            kg, vg = [], []
            for hf in range(2):
                isl = idx_kv[:, 128 * hf : 128 * (hf + 1)]
                kt = gath.tile([128, NS // 2, C], BF16, tag=f"kg{hf}")
                nc.gpsimd.dma_gather(kt[:], ins["kbf"][:], isl, 2048, 2048, C, single_packet=False)
                kg.append(kt)
                vt = gath.tile([128, NS // 2, C], BF16, tag=f"vg{hf}")
                nc.gpsimd.dma_gather(vt[:], ins["vbf"][:], isl, 2048, 2048, C, single_packet=False)
                vg.append(vt)

            # ---- scores ----
            if STAGE < 7:
                continue
            qbf = work.tile([128, C], BF16, tag="qbf")
            nc.vector.tensor_copy(qbf, qr_t)
            sc_raw = work.tile([128, NS], F32, tag="sc_raw")
            for s in range(NS):
                scr = work.tile([128, C], BF16, tag="scr")
                nc.vector.tensor_mul(scr, kg[s // 16][:, s % 16, :], qbf)
                scr2 = work.tile([128, C], BF16, tag="scr2")
                nc.scalar.activation(
                    out=scr2,
                    in_=scr,
                    func=mybir.ActivationFunctionType.Copy,
                    scale=SCALE,
                    accum_out=sc_raw[:, s : s + 1],
                )
            sc_sb = work.tile([128, NS], F32, tag="sc")
            nc.vector.tensor_add(sc_sb, sc_raw, oa_sb[:, NS : 2 * NS])

            # ---- softmax over P within each head ----
            if STAGE < 8:
                continue
            sc3 = sc_sb.rearrange("l (h p) -> l h p", p=P)
            mx = work.tile([128, H], F32, tag="mx")
            nc.vector.tensor_reduce(
                mx, sc3, axis=mybir.AxisListType.X, op=mybir.AluOpType.max
            )
            ex = work.tile([128, H, P], F32, tag="ex")
            nc.vector.tensor_tensor(
                out=ex,
                in0=sc3,
                in1=mx.to_broadcast([128, H, P]),
                op=mybir.AluOpType.subtract,
            )
            exa = work.tile([128, H, P], F32, tag="exa")
            nc.scalar.activation(exa, ex, mybir.ActivationFunctionType.Exp)
            den = work.tile([128, H], F32, tag="den")
            nc.vector.tensor_reduce(
                den, exa, axis=mybir.AxisListType.X, op=mybir.AluOpType.add
            )
            rden = work.tile([128, H], F32, tag="rden")
            nc.vector.reciprocal(rden, den)
            wv = work.tile([128, H, P], F32, tag="wv")
            nc.vector.tensor_tensor(
                out=wv,
                in0=exa,
                in1=rden.to_broadcast([128, H, P]),
                op=mybir.AluOpType.mult,
            )
            wfl = wv.rearrange("l h p -> l (h p)")

            # ---- weighted value sum -> transposed attn chunks ----
            if STAGE < 9:
                continue
            if t % G == 0:
                attnT = att.tile([128, H, KC, G, 128], BF16, tag="attnT")
            tg = t % G
            for h in range(H):
                at_ps = psum.tile([128, KC * 128], F32, tag="at", space="PSUM")
                diags = []
                for p in range(P):
                    s = h * P + p
                    dg = work.tile([128, 128], BF16, tag=f"diag{p}")
                    nc.vector.tensor_scalar_mul(dg, id_sb, wfl[:, s : s + 1])
                    diags.append(dg)
                for cc in range(KC):
                    for p in range(P):
                        s = h * P + p
                        nc.tensor.matmul(
                            at_ps[:, 128 * cc : 128 * (cc + 1)],
                            vg[s // 16][:, s % 16, 128 * cc : 128 * (cc + 1)],
                            diags[p],
                            start=(p == 0),
                            stop=(p == P - 1),
                        )
                nc.scalar.activation(
                    attnT[:, h, :, tg, :],
                    at_ps.rearrange("c (cc l) -> c cc l", l=128),
                    mybir.ActivationFunctionType.Copy,
                )

            # ---- proj for completed group ----
            if STAGE < 10 or t % G != G - 1:
                continue
            if True:
                g0 = t - (G - 1)
                for co in range(KC):
                    pj = psum.tile([128, G * 128], F32, tag="pj", space="PSUM")
                    ch = 0
                    for h in range(H):
                        for cc in range(KC):
                            nc.tensor.matmul(
                                pj,
                                wp_sb[:, h * KC + cc, 128 * co : 128 * (co + 1)],
                                attnT[:, h, cc, :, :],
                                start=(ch == 0),
                                stop=(ch == NPC - 1),
                            )
                            ch += 1
                    yo = work.tile([128, G * 128], F32, tag="yo")
                    nc.scalar.activation(
                        yo,
                        pj,
                        mybir.ActivationFunctionType.Identity,
                        bias=bp_sb[:, co : co + 1],
                    )
                    nc.sync.dma_start(
                        out=yt[
                            128 * co : 128 * (co + 1),
                            128 * g0 : 128 * (g0 + G),
                        ],
                        in_=yo,
                    )


def _rewrap(nc, work, dram, src16, name):
    """[128(l), NS(s)] i16 -> [128, 2*NS*4] i16 wrapped for dma_gather.

    idx[16g+pp, l8 + 8*s] = src[pp + 16*l8, s], replicated over g in 0..7.
    Source partition p = pp + 16*l8 iterates l8-slow/pp-fast as p ascends.
    """
    scratch = dram.tile([128 * NS], I16, tag=f"scr_{name}")
    sap = scratch[:]
    w = bass.AP(
        tensor=sap.tensor,
        offset=sap.offset,
        ap=[[1, 8], [NS * 8, 16], [8, NS]],
    )
    nc.sync.dma_start(out=w, in_=src16)
    idx = work.tile([128, 128 * NS // 16], I16, tag=f"idx_{name}")
    r = bass.AP(
        tensor=sap.tensor, offset=sap.offset, ap=[[0, 8], [1, 128 * NS]]
    )
    nc.gpsimd.dma_start(out=idx, in_=r)
    return idx


@functools.lru_cache(maxsize=1)
def _get_nc():
    return _build_program()


# ----------------------------------------------------------------------------
# host wrapper
# ----------------------------------------------------------------------------
def kernel(query, key, value, W_off, b_off, W_attn, b_attn, W_proj, b_proj,
           num_heads, num_points, _debug=False):
    query = np.asarray(query, np.float32)
    key = np.asarray(key, np.float32)
    value = np.asarray(value, np.float32)
    W_off = np.asarray(W_off, np.float32)
    b_off = np.asarray(b_off, np.float32)
    W_attn = np.asarray(W_attn, np.float32)
    b_attn = np.asarray(b_attn, np.float32)
    W_proj = np.asarray(W_proj, np.float32)
    b_proj = np.asarray(b_proj, np.float32)

    _, win = _thr_window_table()
    nc = _get_nc()

    woa = np.ascontiguousarray(np.concatenate([W_off, W_attn], axis=1))
    boa = np.concatenate([b_off, b_attn])[None, :].copy()
    wp = np.ascontiguousarray(W_proj.astype(_BF16))
    bp = b_proj[None, :].copy()
    ident = np.eye(128, dtype=np.float32).astype(_BF16)

    kbf = [np.ascontiguousarray(key[b].astype(_BF16)) for b in range(B)]
    vbf = [np.ascontiguousarray(value[b].astype(_BF16)) for b in range(B)]

    in_maps = []
    for c in range(NCORES):
        b, hf = c // 2, c % 2
        qs = query[b, hf * LQC : (hf + 1) * LQC]  # [1024, 512]
        in_maps.append(
            dict(
                qt=np.ascontiguousarray(qs.T),
                qr=np.ascontiguousarray(qs),
                kbf=kbf[b],
                vbf=vbf[b],
                woa=woa,
                boa=boa,
                wp=wp,
                bp=bp,
                thr=win,
                ident=ident,
            )
        )

    res = run_bass_kernel_spmd(nc, in_maps, core_ids=list(range(NCORES)))
    globals()["LAST_RES"] = res
    out = np.empty((B, LQ, C), np.float32)
    for c in range(NCORES):
        b, hf = c // 2, c % 2
        out[b, hf * LQC : (hf + 1) * LQC] = np.asarray(
            res.results[c]["yt"], np.float32
        ).T
    if _debug:
        dbg = {
            "loc": np.stack(
                [
                    np.asarray(res.results[c]["dbg_loc"]).reshape(LQC, NS)
                    for c in range(NCORES)
                ]
            ),
            "oa": np.stack(
                [
                    np.asarray(res.results[c]["dbg_oa"]).reshape(LQC, 2 * NS)
                    for c in range(NCORES)
                ]
            ),
        }
        return out, dbg
    return out


# revision 19
# speedup vs baseline: 215.8417x; 215.8417x over previous
"""Trainium2 Bass kernel for nn_DeformableCrossAttention_79551384257136.

Self-contained: takes FULL unsharded inputs, returns FULL output.

Sharding: data-parallel over (batch, query-half): core c handles batch c//2,
query rows [(c%2)*1024, (c%2+1)*1024). Each core sees its batch's full
key/value (4096 x 512) and produces yT [512, 1024] (host re-transposes).

Per-core algorithm:
  1. off/attn GEMM in fp32 on PE (qT chunks as lhsT).
  2. loc indices: ACT sigmoid -> *4095 -> trunc, then corrected EXACTLY to the
     reference's float32 semantics via a host-precomputed threshold table
     (bisected boundaries of m <= sigmoid(x)*4095), gathered with dma_gather
     and a +-15 comparison window.
  3. Sampled K/V rows gathered in bf16 with dma_gather (32 rows/query).
  4. scores: fused DVE tensor_tensor_reduce (q . k * scale + attn_logit).
  5. softmax over the 4 points per head (DVE/ACT, fp32).
  6. Weighted value sum as PE diag-matmuls: psum[c, l] += Vg_p.T @ diag(w_p),
     giving attn_out TRANSPOSED for the proj GEMM.
  7. proj GEMM in bf16 on PE -> yT (+bias), DMA out.
"""

import functools
import os

import numpy as np

import concourse.bass as bass
import concourse.mybir as mybir
import concourse.tile as tile
from concourse import bacc
from concourse.bass_utils import run_bass_kernel_spmd
from concourse.library_config import mlp as _mlp_lib

try:
    import ml_dtypes

    _BF16 = ml_dtypes.bfloat16
except ImportError:  # pragma: no cover
    _BF16 = np.dtype("bfloat16")

F32 = mybir.dt.float32
BF16 = mybir.dt.bfloat16
I16 = mybir.dt.int16

# problem shape (hardcoded per contract)
B, LQ, LK, C, H, P = 4, 2048, 4096, 512, 8, 4
NS = H * P  # 32 sample slots per query
NCORES = 8
LQC = LQ // 2  # queries per core (1024)
NT = LQC // 128  # Lq tiles per core (8)
G = 2  # tiles per proj group
KC = C // 128  # contraction chunks for C (4)
NPC = (C * H) // 128  # proj contraction chunks (32)
WIN = 16  # threshold window entries used (of each 64-entry row)
SCALE = 1.0 / float(np.sqrt(np.float32(C)))
LAST_RES = None


# ----------------------------------------------------------------------------
# threshold table: T[m] = min float32 x with the reference index >= m,
# computed with the same jax that executes the reference. Input-independent.
# ----------------------------------------------------------------------------
def _f32_key(x):
    b = np.asarray(x, np.float32).view(np.uint32).astype(np.int64)
    neg = b >= 0x80000000
    return np.where(neg, 0xFFFFFFFF - b, b + 0x80000000)


def _key_f32(k):
    k = np.asarray(k, np.int64)
    neg = k < 0x80000000
    b = np.where(neg, 0xFFFFFFFF - k, k - 0x80000000).astype(np.uint32)
    return b.view(np.float32)


@functools.lru_cache(maxsize=1)
def _thr_window_table():
    cache = os.path.join(
        os.path.dirname(os.path.abspath(__file__)), f".thr_cache_{LK}.npy"
    )
    T = None
    if os.path.exists(cache):
        try:
            T = np.load(cache)
        except Exception:
            T = None
    if T is None or T.shape != (LK,):
        import jax
        import jax.numpy as jnp

        def _idx(x):
            # eager, op-by-op: must match reference.py's eager dispatch
            # numerics exactly (a fused jit compiles differently here).
            s = jax.nn.sigmoid(x)
            t = s * (LK - 1)
            n = t.astype(jnp.int32)
            return jnp.clip(n, 0, LK - 1)

        m = np.arange(1, LK, dtype=np.int64)  # 1..4095
        lo = np.full(m.shape, _f32_key(np.float32(-40.0)), np.int64)
        hi = np.full(m.shape, _f32_key(np.float32(40.0)), np.int64)
        # invariant: idx(inv(hi)) >= m, idx(inv(lo)) < m
        while True:
            if ((hi - lo) <= 1).all():
                break
            mid = (lo + hi) // 2
            v = np.asarray(_idx(jnp.asarray(_key_f32(mid)))).astype(np.int64)
            ge = v >= m
            hi = np.where(ge, mid, hi)
            lo = np.where(ge, lo, mid)
        T = np.concatenate(
            [[np.float32(-1e30)], _key_f32(hi)]
        ).astype(np.float32)  # T[0] = -inf sentinel, T[1..4095]
        try:
            np.save(cache, T)
        except Exception:
            pass
    # window table: win[m, j] = T_pad[m - 15 + j], j in 0..31; cols 32..63 pad
    Tpad = np.concatenate(
        [
            np.full(7, np.float32(-1e30), np.float32),
            T.astype(np.float32),
            np.full(9, np.float32(1e30), np.float32),
        ]
    )
    idx = np.arange(LK)[:, None] + np.arange(WIN)[None, :]
    win = np.zeros((LK, 64), np.float32)
    win[:, :WIN] = Tpad[idx]
    return T, win


def _searchsorted_loc(x):
    """Reference-exact loc from off values using the threshold table."""
    T, _ = _thr_window_table()
    return np.searchsorted(T[1:], np.asarray(x, np.float32), side="right").astype(
        np.int32
    )


# ----------------------------------------------------------------------------
# bass program (built once; SPMD across 8 cores via per-core in_maps)
# ----------------------------------------------------------------------------
def _build_program():
    nc = bacc.Bacc(
        "TRN2", target_bir_lowering=False, debug=False, num_devices=NCORES
    )
    dt_in = dict(
        qt=([C, LQC], F32),
        qr=([LQC, C], F32),
        kvbf=([LK, 2 * C], BF16),
        woa=([C, 2 * NS], F32),
        boa=([1, 2 * NS], F32),
        wp=([C * H, C], BF16),
        bp=([1, C], F32),
        thr=([LK, 64], F32),
        ident=([128, 128], BF16),
    )
    ins = {
        k: nc.dram_tensor(k, s, d, kind="ExternalInput").ap()
        for k, (s, d) in dt_in.items()
    }
    yt = nc.dram_tensor("yt", [C, LQC], F32, kind="ExternalOutput").ap()
    dbg_loc = nc.dram_tensor(
        "dbg_loc", [NT, 128, NS], I16, kind="ExternalOutput"
    ).ap()
    dbg_oa = nc.dram_tensor(
        "dbg_oa", [NT, 128, 2 * NS], F32, kind="ExternalOutput"
    ).ap()

    with tile.TileContext(nc) as tc:
        _body(tc, ins, yt, dbg_loc, dbg_oa)
    nc.compile()
    return nc


def _body(tc, ins, yt, dbg_loc, dbg_oa):
    from contextlib import ExitStack

    nc = tc.nc
    with ExitStack() as ctx:
        const = ctx.enter_context(tc.tile_pool(name="const", bufs=1))
        work = ctx.enter_context(tc.tile_pool(name="work", bufs=2))
        gath = ctx.enter_context(tc.tile_pool(name="gath", bufs=1))
        thrp = ctx.enter_context(tc.tile_pool(name="thrp", bufs=2))
        att = ctx.enter_context(tc.tile_pool(name="att", bufs=2))
        psum = ctx.enter_context(tc.tile_pool(name="psum", bufs=2, space="PSUM"))
        pjp = ctx.enter_context(tc.tile_pool(name="pjp", bufs=1, space="PSUM"))
        dram = ctx.enter_context(tc.tile_pool(name="dram", bufs=3, space="DRAM"))

        nc.gpsimd.load_library(_mlp_lib)

        # ---- constants ----
        woa_sb = const.tile([128, KC, 2 * NS], F32)
        nc.sync.dma_start(
            out=woa_sb, in_=ins["woa"].rearrange("(k p) n -> p k n", p=128)
        )
        boa_sb = const.tile([128, 2 * NS], F32)
        nc.sync.dma_start(
            out=boa_sb,
            in_=bass.AP(
                tensor=ins["boa"].tensor, offset=0, ap=[[0, 128], [1, 2 * NS]]
            ),
        )
        wp_sb = const.tile([128, NPC, C], BF16)
        nc.sync.dma_start(
            out=wp_sb, in_=ins["wp"].rearrange("(k p) n -> p k n", p=128)
        )
        bp_sb = const.tile([128, KC], F32)
        nc.sync.dma_start(
            out=bp_sb,
            in_=bass.AP(
                tensor=ins["bp"].tensor, offset=0, ap=[[1, 128], [128, KC]]
            ),
        )
        id_sb = const.tile([128, 128], BF16)
        nc.sync.dma_start(out=id_sb, in_=ins["ident"])

        def stage_a(t):
            """Index pipeline for tile t -> (qr_t, oa_sb, idx_kv)."""
            qt_t = work.tile([128, KC, 128], F32, tag="qt")
            nc.sync.dma_start(
                out=qt_t,
                in_=ins["qt"].rearrange("(k p) l -> p k l", p=128)[
                    :, :, 128 * t : 128 * (t + 1)
                ],
            )
            qr_t = work.tile([128, C], F32, tag="qr")
            nc.sync.dma_start(out=qr_t, in_=ins["qr"][128 * t : 128 * (t + 1), :])

            oa_ps = psum.tile([128, 2 * NS], F32, tag="oa", space="PSUM")
            for k in range(KC):
                nc.tensor.matmul(
                    oa_ps,
                    qt_t[:, k, :],
                    woa_sb[:, k, :],
                    start=(k == 0),
                    stop=(k == KC - 1),
                )
            oa_sb = work.tile([128, 2 * NS], F32, tag="oa_sb")
            nc.vector.tensor_add(oa_sb, oa_ps, boa_sb)
            nc.sync.dma_start(out=dbg_oa[t], in_=oa_sb)

            # sigmoid via exp (keeps every ACT func in the exp table set)
            eneg = work.tile([128, NS], F32, tag="eneg")
            nc.scalar.activation(
                eneg,
                oa_sb[:, 0:NS],
                mybir.ActivationFunctionType.Exp,
                scale=-1.0,
            )
            d1 = work.tile([128, NS], F32, tag="d1")
            nc.vector.tensor_scalar_add(d1, eneg, 1.0)
            sg = work.tile([128, NS], F32, tag="sg")
            nc.vector.reciprocal(sg, d1)
            tt = work.tile([128, NS], F32, tag="tt")
            nc.vector.tensor_scalar_mul(tt, sg, float(LK - 1))
            nh16 = work.tile([128, NS], I16, tag="nh16")
            nc.vector.tensor_copy(nh16, tt)
            nhf = work.tile([128, NS], F32, tag="nhf")
            nc.vector.tensor_copy(nhf, nh16)

            idx_thr = _rewrap(nc, work, dram, nh16, "thr")
            thr_sb = thrp.tile([128, NS, 64], F32, tag="thrw")
            nc.gpsimd.dma_gather(
                thr_sb[:],
                ins["thr"][:],
                idx_thr[:],
                128 * NS,
                128 * NS,
                64,
                single_packet=False,
            )
            return qr_t, oa_sb, thr_sb, nhf

        def stage_a_post(t, oa_sb, thr_sb, nhf):
            ge = work.tile([128, NS, WIN], F32, tag="ge")
            nc.vector.tensor_tensor(
                out=ge,
                in0=thr_sb[:, :, 0:WIN],
                in1=oa_sb[:, 0:NS].to_broadcast([128, NS, WIN]),
                op=mybir.AluOpType.is_le,
            )
            wsum = work.tile([128, NS], F32, tag="wsum")
            nc.vector.tensor_reduce(
                wsum, ge, axis=mybir.AxisListType.X, op=mybir.AluOpType.add
            )
            nf = work.tile([128, NS], F32, tag="nf")
            nc.vector.tensor_scalar_add(nf, wsum, -8.0)
            nc.vector.tensor_add(nf, nf, nhf)
            n16 = work.tile([128, NS], I16, tag="n16")
            nc.vector.tensor_copy(n16, nf)
            nc.sync.dma_start(out=dbg_loc[t], in_=n16)

            idx_kv = _rewrap(nc, work, dram, n16, "kv")
            return idx_kv

        def stage_b_gather(t, idx_kv):
            kv = []
            for q in range(4):
                isl = idx_kv[:, 64 * q : 64 * (q + 1)]
                kt = gath.tile(
                    [128, NS // 4, 2 * C], BF16, tag=f"kv{q % 2}"
                )
                nc.gpsimd.dma_gather(
                    kt[:],
                    ins["kvbf"][:],
                    isl,
                    1024,
                    1024,
                    2 * C,
                    single_packet=False,
                )
                kv.append(kt)
            return kv

        def quarter_scores(t, q, kvq_t, qbf, oa_sb):
            """Slots 8q..8q+7 (heads 2q, 2q+1): scores + softmax -> weights."""
            b0 = 8 * q
            sc_raw = work.tile([128, 8], F32, tag="sc_raw")
            if q == 0:
                # DVE path: per-slot reduce
                for j in range(8):
                    scr = work.tile([128, C], BF16, tag="scr")
                    nc.vector.tensor_mul(scr, kvq_t[:, j, 0:C], qbf)
                    nc.vector.tensor_reduce(
                        sc_raw[:, j : j + 1],
                        scr,
                        axis=mybir.AxisListType.X,
                        op=mybir.AluOpType.add,
                    )
                nc.vector.tensor_scalar_mul(sc_raw, sc_raw, SCALE)
            else:
                # ACT path: accum-copy with fused scale
                for j in range(8):
                    scr = work.tile([128, C], BF16, tag="scr")
                    nc.vector.tensor_mul(scr, kvq_t[:, j, 0:C], qbf)
                    scr2 = work.tile([128, C], BF16, tag="scr2")
                    nc.scalar.activation(
                        out=scr2,
                        in_=scr,
                        func=mybir.ActivationFunctionType.Copy,
                        scale=SCALE,
                        accum_out=sc_raw[:, j : j + 1],
                    )
            scq = work.tile([128, 8], F32, tag="scq")
            nc.vector.tensor_add(
                scq, sc_raw, oa_sb[:, NS + b0 : NS + b0 + 8]
            )
            sc3 = scq.rearrange("l (h p) -> l h p", p=P)
            mx = work.tile([128, 2], F32, tag="mx")
            nc.vector.tensor_reduce(
                mx, sc3, axis=mybir.AxisListType.X, op=mybir.AluOpType.max
            )
            ex = work.tile([128, 2, P], F32, tag="ex")
            nc.vector.tensor_tensor(
                out=ex,
                in0=sc3,
                in1=mx.to_broadcast([128, 2, P]),
                op=mybir.AluOpType.subtract,
            )
            exa = work.tile([128, 2, P], F32, tag="exa")
            nc.scalar.activation(exa, ex, mybir.ActivationFunctionType.Exp)
            den = work.tile([128, 2], F32, tag="den")
            nc.vector.tensor_reduce(
                den, exa, axis=mybir.AxisListType.X, op=mybir.AluOpType.add
            )
            rden = work.tile([128, 2], F32, tag="rden")
            nc.vector.reciprocal(rden, den)
            wv = work.tile([128, 2, P], F32, tag="wv")
            nc.vector.tensor_tensor(
                out=wv,
                in0=exa,
                in1=rden.to_broadcast([128, 2, P]),
                op=mybir.AluOpType.mult,
            )
            return wv.rearrange("l h p -> l (h p)")

        def quarter_wsum(t, q, kvq_t, wfl, state):
            tg = t % G
            if tg == 0 and q == 0:
                attnT_t = att.tile([128, H, KC, G, 128], BF16, tag="attnT")
                state["attnT"] = attnT_t
                state["pj"] = None
            attnT = state["attnT"]
            if tg == G - 1 and state["pj"] is None:
                pj_tiles = []
                for co in range(KC):
                    pj_t = pjp.tile(
                        [128, G * 128], F32, tag=f"pj{co}", space="PSUM"
                    )
                    pj_tiles.append(pj_t)
                state["pj"] = pj_tiles
            for hh in range(2):
                h = 2 * q + hh
                at_ps = psum.tile([128, KC * 128], F32, tag="at", space="PSUM")
                diags = []
                for p in range(P):
                    j = hh * P + p
                    dg = work.tile([128, 128], BF16, tag=f"diag{p}")
                    nc.vector.tensor_scalar_mul(dg, id_sb, wfl[:, j : j + 1])
                    diags.append(dg)
                for cc in range(KC):
                    for p in range(P):
                        j = hh * P + p
                        nc.tensor.matmul(
                            at_ps[:, 128 * cc : 128 * (cc + 1)],
                            kvq_t[:, j, C + 128 * cc : C + 128 * (cc + 1)],
                            diags[p],
                            start=(p == 0),
                            stop=(p == P - 1),
                        )
                nc.scalar.activation(
                    attnT[:, h, :, tg, :],
                    at_ps.rearrange("c (cc l) -> c cc l", l=128),
                    mybir.ActivationFunctionType.Copy,
                )
                if tg == G - 1:
                    pj = state["pj"]
                    for cc in range(KC):
                        for co in range(KC):
                            nc.tensor.matmul(
                                pj[co],
                                wp_sb[:, h * KC + cc, 128 * co : 128 * (co + 1)],
                                attnT[:, h, cc, :, :],
                                start=(h == 0 and cc == 0),
                                stop=(h == H - 1 and cc == KC - 1),
                            )
            if tg == G - 1 and q == 3:
                g0 = t - (G - 1)
                for co in range(KC):
                    yo = work.tile([128, G * 128], F32, tag="yo")
                    nc.scalar.activation(
                        yo,
                        state["pj"][co],
                        mybir.ActivationFunctionType.Identity,
                        bias=bp_sb[:, co : co + 1],
                    )
                    nc.sync.dma_start(
                        out=yt[
                            128 * co : 128 * (co + 1),
                            128 * g0 : 128 * (g0 + G),
                        ],
                        in_=yo,
                    )

        state = {}
        pre = {0: stage_a(0)}
        kvq = {
            0: stage_b_gather(
                0, stage_a_post(0, pre[0][1], pre[0][2], pre[0][3])
            )
        }
        if NT > 1:
            pre[1] = stage_a(1)
        for t in range(NT):
            kv = kvq.pop(t)
            qr_t, oa_sb = pre[t][0], pre[t][1]
            qbf = work.tile([128, C], BF16, tag="qbf")
            nc.vector.tensor_copy(qbf, qr_t)
            for q in range(4):
                wfl = quarter_scores(t, q, kv[q], qbf, oa_sb)
                quarter_wsum(t, q, kv[q], wfl, state)
                if q == 1 and t + 1 < NT:
                    p = pre[t + 1]
                    idx_kv = stage_a_post(t + 1, p[1], p[2], p[3])
                    kvq[t + 1] = stage_b_gather(t + 1, idx_kv)
                if q == 2 and t + 2 < NT:
                    pre[t + 2] = stage_a(t + 2)
            pre.pop(t)


def _rewrap(nc, work, dram, src16, name):
    """[128(l), NS(s)] i16 -> [128, 2*NS*4] i16 wrapped for dma_gather.

    idx[16g+pp, l8 + 8*s] = src[pp + 16*l8, s], replicated over g in 0..7.
    Source partition p = pp + 16*l8 iterates l8-slow/pp-fast as p ascends.
    """
    scratch = dram.tile([128 * NS], I16, tag=f"scr_{name}")
    sap = scratch[:]
    w = bass.AP(
        tensor=sap.tensor,
        offset=sap.offset,
        ap=[[1, 8], [NS * 8, 16], [8, NS]],
    )
    nc.sync.dma_start(out=w, in_=src16)
    idx = work.tile([128, 128 * NS // 16], I16, tag=f"idx_{name}")
    r = bass.AP(
        tensor=sap.tensor, offset=sap.offset, ap=[[0, 8], [1, 128 * NS]]
    )
    nc.sync.dma_start(out=idx, in_=r)
    return idx


@functools.lru_cache(maxsize=1)
def _get_nc():
    return _build_program()


# ----------------------------------------------------------------------------
# host wrapper
# ----------------------------------------------------------------------------
def kernel(query, key, value, W_off, b_off, W_attn, b_attn, W_proj, b_proj,
           num_heads, num_points, _debug=False):
    query = np.asarray(query, np.float32)
    key = np.asarray(key, np.float32)
    value = np.asarray(value, np.float32)
    W_off = np.asarray(W_off, np.float32)
    b_off = np.asarray(b_off, np.float32)
    W_attn = np.asarray(W_attn, np.float32)
    b_attn = np.asarray(b_attn, np.float32)
    W_proj = np.asarray(W_proj, np.float32)
    b_proj = np.asarray(b_proj, np.float32)

    _, win = _thr_window_table()
    nc = _get_nc()

    woa = np.ascontiguousarray(np.concatenate([W_off, W_attn], axis=1))
    boa = np.concatenate([b_off, b_attn])[None, :].copy()
    wp = np.ascontiguousarray(W_proj.astype(_BF16))
    bp = b_proj[None, :].copy()
    ident = np.eye(128, dtype=np.float32).astype(_BF16)

    kvbf = [
        np.ascontiguousarray(
            np.concatenate(
                [key[b].astype(_BF16), value[b].astype(_BF16)], axis=1
            )
        )
        for b in range(B)
    ]

    in_maps = []
    for c in range(NCORES):
        b, hf = c // 2, c % 2
        qs = query[b, hf * LQC : (hf + 1) * LQC]  # [1024, 512]
        in_maps.append(
            dict(
                qt=np.ascontiguousarray(qs.T),
                qr=np.ascontiguousarray(qs),
                kvbf=kvbf[b],
                woa=woa,
                boa=boa,
                wp=wp,
                bp=bp,
                thr=win,
                ident=ident,
            )
        )

    res = run_bass_kernel_spmd(nc, in_maps, core_ids=list(range(NCORES)))
    globals()["LAST_RES"] = res
    out = np.empty((B, LQ, C), np.float32)
    for c in range(NCORES):
        b, hf = c // 2, c % 2
        out[b, hf * LQC : (hf + 1) * LQC] = np.asarray(
            res.results[c]["yt"], np.float32
        ).T
    if _debug:
        dbg = {
            "loc": np.stack(
                [
                    np.asarray(res.results[c]["dbg_loc"]).reshape(LQC, NS)
                    for c in range(NCORES)
                ]
            ),
            "oa": np.stack(
                [
                    np.asarray(res.results[c]["dbg_oa"]).reshape(LQC, 2 * NS)
                    for c in range(NCORES)
                ]
            ),
        }
        return out, dbg
    return out
